# revision 1
# baseline (speedup 1.0000x reference)
"""ClusterHead (vq_codebook) Trainium2 kernel.

Computes softmax(-0.5 * ||x_n - c_k||^2, axis=k) for x [32768, 512],
centers [1024, 512] -> probabilities [32768, 1024] (float32).

Strategy (8 NeuronCores, data-parallel over N):
  - Shard x rows across cores (4096 rows each); replicate centers.
  - Host pre-transposes x-shard and centers to d-major so both matmul
    operands are already [d, *] (the PE contracts along the partition dim).
  - softmax(logit) with logit[n,k] = x.c - 0.5*||c_k||^2 (the per-row
    -0.5*||x_n||^2 term is constant along k and cancels in softmax).
  - The -0.5*||c_k||^2 bias is folded into the matmul as two extra
    contraction rows (ones (x) bias_hi + ones (x) bias_lo; hi carries the
    top 7 mantissa bits so float32r's reduced-precision path loses nothing).
  - Per 128-row tile:
      PE:  2 k-halves x (4 data matmuls + 1 bias matmul) -> PSUM = logits
      DVE: m = -rowmax(logits)            (tensor_reduce max, negate=True)
      ACT: e = exp(logits + m), z = rowsum(e)
      DVE: r = 1/z ; out = e * r
      DMA: store [128, 1024] f32 tile
  - matmul dtype float32r: fp32 data streamed at bf16 rate (1 cyc/row);
    measured cross-product rel err ~1.5e-4 vs fp32.
"""

import numpy as np

import concourse.bass as bass
import concourse.mybir as mybir
import concourse.tile as tile
from concourse import bacc, bass_utils

N_CORES = 8
N, D, K = 32768, 512, 1024
NS = N // N_CORES  # rows per core
P = 128
N_TILES = NS // P  # 32
DB = D // P        # 4 contraction blocks
KH = 512           # matmul free-dim half (fp32 PSUM bank limit)

MM_DT = mybir.dt.float32r


def build_bass(mm_dt=MM_DT):
    # Bacc (not raw Bass): its compile() runs move_matmul_waits_to_ldweights
    # + generate_event_semaphores, which legalize instructions that would
    # otherwise carry more sync-waits than the ISA structs allow.
    nc = bacc.Bacc("TRN2", debug=False, num_devices=N_CORES)

    xT = nc.dram_tensor("xT", [D, NS], mm_dt, kind="ExternalInput").ap()
    cT = nc.dram_tensor("cT", [D, K], mm_dt, kind="ExternalInput").ap()
    ncsq = nc.dram_tensor("ncsq", [2, K], mm_dt, kind="ExternalInput").ap()
    out = nc.dram_tensor("out", [NS, K], mybir.dt.float32, kind="ExternalOutput").ap()

    xT_r = xT.rearrange("(b p) n -> p b n", p=P)  # [128, DB, NS]
    cT_r = cT.rearrange("(b p) k -> p b k", p=P)  # [128, DB, K]

    CHUNK_T = 4
    CHUNK_N = CHUNK_T * P  # 512 rows per x chunk (~1MB DMA)
    with tile.TileContext(nc) as tc:
        with (
            tc.tile_pool(name="singles", bufs=1) as singles,
            tc.tile_pool(name="pss", bufs=1, space="PSUM") as pss,
            tc.tile_pool(name="xp", bufs=3) as xp,
            tc.tile_pool(name="ep", bufs=3) as ep,
            tc.tile_pool(name="outp", bufs=3) as outp,
            tc.tile_pool(name="small", bufs=12) as small,
        ):
            ct_s = singles.tile([P, DB, K], mm_dt)
            nc.sync.dma_start(ct_s, cT_r)
            ncsq_s = singles.tile([2, K], mm_dt)
            nc.sync.dma_start(ncsq_s, ncsq)
            ones_f32 = singles.tile([2, P], mybir.dt.float32)
            nc.vector.memset(ones_f32, 1.0)
            ones_s = ones_f32[:].bitcast(mm_dt)

            psum_all = pss.tile([P, 4, K], mybir.dt.float32)

            for c in range(NS // CHUNK_N):
                xt = xp.tile([P, DB, CHUNK_N], mm_dt)
                nc.sync.dma_start(
                    xt, xT_r[:, :, c * CHUNK_N : (c + 1) * CHUNK_N]
                )

                for i in range(CHUNK_T):
                    nt = c * CHUNK_T + i
                    n0 = nt * P
                    psum = psum_all[:, nt % 4, :]
                    for h in range(2):
                        hs = slice(h * KH, (h + 1) * KH)
                        for kb in range(DB):
                            nc.tensor.matmul(
                                psum[:, hs],
                                lhsT=xt[:, kb, i * P : (i + 1) * P],
                                rhs=ct_s[:, kb, hs],
                                start=(kb == 0),
                                stop=False,
                            )
                        nc.tensor.matmul(
                            psum[:, hs],
                            lhsT=ones_s,
                            rhs=ncsq_s[:, hs],
                            start=False,
                            stop=True,
                        )

                    m = small.tile([P, 1], mybir.dt.float32)
                    nc.vector.tensor_reduce(
                        m,
                        psum,
                        axis=mybir.AxisListType.X,
                        op=mybir.AluOpType.max,
                        negate=True,
                    )
                    e = ep.tile([P, K], mybir.dt.float32)
                    z = small.tile([P, 1], mybir.dt.float32)
                    nc.scalar.activation(
                        out=e,
                        in_=psum,
                        func=mybir.ActivationFunctionType.Exp,
                        bias=m,
                        scale=1.0,
                        accum_out=z,
                    )
                    r = small.tile([P, 1], mybir.dt.float32)
                    nc.vector.reciprocal(r, z)
                    o = outp.tile([P, K], mybir.dt.float32)
                    nc.vector.tensor_scalar_mul(o, e, r)
                    nc.sync.dma_start(out[n0 : n0 + P, :], o)

    nc.compile()
    return nc


def _prep_in_maps(x, centers):
    x = np.ascontiguousarray(np.asarray(x, dtype=np.float32))
    centers = np.ascontiguousarray(np.asarray(centers, dtype=np.float32))
    cT = np.ascontiguousarray(centers.T)
    b = (-0.5 * (centers.astype(np.float64) ** 2).sum(axis=1)).astype(np.float32)
    # hi keeps the top 7 mantissa bits (bf16-truncation) so it is exactly
    # representable under any reduced-precision matmul path; lo is the
    # small remainder.
    hi = (b.view(np.uint32) & np.uint32(0xFFFF0000)).view(np.float32)
    lo = b - hi
    ncsq = np.ascontiguousarray(np.stack([hi, lo], axis=0))  # [2, K]
    in_maps = []
    for c in range(N_CORES):
        xs = x[c * NS : (c + 1) * NS]
        in_maps.append(
            {"xT": np.ascontiguousarray(xs.T), "cT": cT, "ncsq": ncsq}
        )
    return in_maps


def run(x, centers, mm_dt=MM_DT, **run_kwargs):
    """Build, run on 8 cores, return (output, BassKernelResults)."""
    in_maps = _prep_in_maps(x, centers)
    nc = build_bass(mm_dt)
    res = bass_utils.run_bass_kernel_spmd(
        nc, in_maps, core_ids=list(range(N_CORES)), **run_kwargs
    )
    out = np.concatenate([r["out"] for r in res.results], axis=0)
    return out, res


def kernel(x, centers):
    out, _ = run(x, centers)
    return out

